# revision 20
# baseline (speedup 1.0000x reference)
"""BQQ linear inference kernel for 8 Trainium2 NeuronCores.

Factorized form: after (host-side) activation quantization the op is linear
in X_int, and each (j,k) weight block is rank-32 plus rank-2 corrections:

  out[b, (j,m)] = act_scale * sum_k [ Yc_jk^T (Z_jk X_k^T) ]{m,b}  (main)
               + corrections(Sx, out3, out4) + bias

Device kernel per core (j-sharded, 4 of 32 j-blocks):
  stage0  corrections+bias enter PSUM as one 37-row matmul per j block
          (rows: 32 Sx terms, 4 per-j scalars, 1 bias row).
  stage1  T_k[(j,p,l), b] = Zstack_k^T @ X_k^T   (one 128x512 MM per k)
  copy    T_k PSUM -> SBUF fp16, split across vector/scalar engines
  stage2  outp_j[m, b] += Yc_jk^T @ T_k[32j:32j+32]  (4 row-tiled rank-32
          MMs per k, tile_position=(32j,0), software-pipelined one k behind
          stage1 so the PE never waits on the copies)
  out     PSUM -> SBUF bf16, DMA out as [jm, b]; host transposes.

x ships int8 (2MB) and is upcast to bf16 on the otherwise idle gpsimd
engine.  Per-core HBM traffic ~5.6 MB; PE streams ~half the dense GEMM.
"""

import numpy as np
import ml_dtypes

import concourse.bass as bass
import concourse.bacc as bacc
import concourse.mybir as mybir
from concourse.tile import TileContext
from concourse.tile_rust import add_dep_helper
from concourse.bass_utils import run_bass_kernel_spmd

F32 = mybir.dt.float32
BF16 = mybir.dt.bfloat16
FP16 = mybir.dt.float16
I8 = mybir.dt.int8

P_, J, K, M, L, N = 2, 32, 32, 128, 16, 128
B = 512                  # tokens
NCORES = 8
JLOC = J // NCORES       # 4 j-blocks per core
CPJ = JLOC * M           # 512 output rows per core ((j,m) major)
QMAX = 127.0
UROWS = 37               # 32 Sx rows + 4 corr rows + 1 bias row
HEADK = 12               # first k-slices shipped as ready bf16 (no upcast)
WARMUP = 72

_CACHE = {}


def _build_bass():
    nc = bacc.Bacc()
    xt_d = nc.declare_dram_parameter("xt8", [N, K * B], I8, isOutput=False)
    xh_d = nc.declare_dram_parameter("xth", [N, HEADK * B], BF16,
                                     isOutput=False)
    z_d = nc.declare_dram_parameter("zt", [N, K * 128], BF16, isOutput=False)
    y_d = nc.declare_dram_parameter("yt", [128, K * M], FP16, isOutput=False)
    u_d = nc.declare_dram_parameter("ut", [UROWS, JLOC * M], FP16,
                                    isOutput=False)
    r_d = nc.declare_dram_parameter("rt", [UROWS, B], FP16, isOutput=False)
    out_d = nc.declare_dram_parameter("out", [CPJ, B], BF16, isOutput=True)

    with TileContext(nc) as tc:
        with tc.tile_pool(name="big", bufs=1) as big, \
             tc.tile_pool(name="sm", bufs=1) as sm, \
             tc.tile_pool(name="ot", bufs=4) as ot, \
             tc.tile_pool(name="psum", bufs=1, space="PSUM") as pp:
            xi8 = big.tile([N, K * B], I8)        # x^T int8
            xbt = big.tile([N, K * B], BF16)      # x^T upcast to bf16
            ztt = big.tile([N, K * 128], BF16)    # Z sign stacks
            ytt = big.tile([128, K * M], FP16)    # scaled Y stacks
            tsb = [big.tile([128, B], FP16, name=f'tsb{i}') for i in range(3)]
            utt = sm.tile([UROWS, JLOC * M], FP16)
            rtt = sm.tile([UROWS, B], FP16)
            wz = sm.tile([128, 192], BF16)        # zeros for PE warmup
            wzms = nc.vector.memset(wz[:], 0.0)

            tp = [pp.tile([128, B], F32, name=f"tps{i}", tag=f"tps{i}")
                  for i in range(3)]
            outp = [pp.tile([128, B], F32, name=f"op{i}", tag=f"op{i}")
                    for i in range(4)]
            wps = pp.tile([128, 64], F32, name="wps", tag="wps")

            # Phase A: k-ordered streaming.  x^T int8 on the sync HWDGE
            # ring; Z/Y stacks on the scalar ring; correction matrices
            # first (they open the PSUM accumulation groups).  Slim dummy
            # matmuls paced by the first DMA warm the PE HAM clock.
            # All input DMAs issue from the (otherwise idle) sync engine in
            # one k-ordered sequence; each trigger costs ~650ns of engine
            # time, so keeping them off vector/scalar matters.
            def dx_h(ka, kb):      # bf16 head slices of x
                nc.sync.dma_start(out=xbt[:, ka * B:kb * B],
                                  in_=xh_d[:, ka * B:kb * B])
            def dx_i(ka, kb):      # int8 slices of x
                nc.sync.dma_start(out=xi8[:, ka * B:kb * B],
                                  in_=xt_d[:, ka * B:kb * B])
            def dz(ka, kb):
                nc.gpsimd.dma_start(out=ztt[:, ka * 128:kb * 128],
                                  in_=z_d[:, ka * 128:kb * 128])
            def dy(ka, kb):
                nc.gpsimd.dma_start(out=ytt[:, ka * M:kb * M],
                                  in_=y_d[:, ka * M:kb * M])
            nc.sync.dma_start(out=utt[:], in_=u_d[:])
            nc.sync.dma_start(out=rtt[:], in_=r_d[:])
            dz(0, 8); dy(0, 8)
            dx_h(0, 2); dx_h(2, 6); dx_h(6, 12)
            dz(8, 16); dy(8, 16)
            dx_i(12, 16); dx_i(16, 20)
            dz(16, 24); dy(16, 24)
            dx_i(20, 24); dx_i(24, 28)
            dz(24, 32); dy(24, 32)
            dx_i(28, 32)
            # PE warmup paced off the wz memset so it runs during the DMA
            # wait and hands the HAM clock over warm.
            for w in range(WARMUP):
                mm = nc.tensor.matmul(
                    wps[:], lhsT=wz[:, 0:128],
                    rhs=wz[:, 128:192], start=True, stop=True)
                add_dep_helper(mm.ins, wzms.ins,
                               reason="pace PE warmup after wz memset")

            # x upcast int8 -> bf16 on DVE for k >= HEADK (the scalar
            # engine faults on int8 input; gpsimd is ~10x too slow)
            def upcast(k):
                nc.vector.tensor_copy(out=xbt[:, k * B:(k + 1) * B],
                                      in_=xi8[:, k * B:(k + 1) * B])

            # Phase B: corrections open the output accumulation groups.
            for j in range(JLOC):
                nc.tensor.matmul(
                    outp[j][:], lhsT=utt[:, j * M:(j + 1) * M], rhs=rtt[:],
                    start=True, stop=False)

            def s1(k):
                nc.tensor.matmul(
                    tp[k % 3][:], lhsT=ztt[:, k * 128:(k + 1) * 128],
                    rhs=xbt[:, k * B:(k + 1) * B], start=True, stop=True)

            def tcopy(k):
                t = tsb[k % 3]
                nc.vector.tensor_copy(out=t[:, 0:128], in_=tp[k % 3][:, 0:128])
                nc.scalar.copy(t[:, 128:512], tp[k % 3][:, 128:512])

            def s2(k):
                for j in range(JLOC):
                    nc.tensor.matmul(
                        outp[j][:],
                        lhsT=ytt[32 * j:32 * (j + 1), k * M:(k + 1) * M],
                        rhs=tsb[k % 3][32 * j:32 * (j + 1), :],
                        start=False, stop=(k == K - 1),
                        tile_position=(32 * j, 0))

            s1(0)
            tcopy(0)
            s1(1)
            tcopy(1)
            for k in range(2, K):
                s1(k)
                s2(k - 2)
                tcopy(k)
                ku = k + 4
                if HEADK <= ku < K:
                    upcast(ku)
            s2(K - 2)
            s2(K - 1)

            # Phase C: PSUM -> SBUF bf16 (split engines), DMA out rows
            # (j,m); the host transposes to [b, (j,m)].
            for j in range(JLOC):
                o = ot.tile([128, B], BF16)
                nc.vector.tensor_copy(out=o[:, 0:256], in_=outp[j][:, 0:256])
                nc.scalar.copy(o[:, 256:512], outp[j][:, 256:512])
                eng = nc.sync if j % 2 == 0 else nc.scalar
                eng.dma_start(out=out_d[j * 128:(j + 1) * 128, :], in_=o[:])
    return nc


def _prepare(inputs):
    x = np.asarray(inputs["input"], dtype=np.float32)
    Ys = np.asarray(inputs["Y_sign"], np.float32)
    Zs = np.asarray(inputs["Z_sign"], np.float32)
    ysc = np.asarray(inputs["Y_scale"], np.float32)[..., 0, 0]
    zsc = np.asarray(inputs["Z_scale"], np.float32)[..., 0, 0]
    A = np.asarray(inputs["A"], np.float32)
    bias = np.asarray(inputs["bias"], np.float32)
    a0, a1, a2, a3 = A[..., 0], A[..., 1], A[..., 2], A[..., 3]

    # activation quantization on host (exact global max/min, RNE round)
    act_scale = max((float(x.max()) - float(x.min())) / (2.0 * QMAX), 1e-8)
    Xi = np.clip(np.round(x.reshape(B, K * N) / act_scale), -QMAX, QMAX)
    Xkn = Xi.reshape(B, K, N)
    Sx = Xkn.sum(-1)                                   # [B,K] (exact ints)

    c = a0 * ysc * zsc * act_scale                     # [P,J,K]
    B2 = np.einsum('pjk,pjkm->jkm', a1 * ysc, Ys.sum(-1)) * act_scale
    C3 = np.einsum('pjk,pjkn->jkn', a2 * zsc, Zs.sum(-2))
    out3 = np.einsum('bkn,jkn->bj', Xkn, C3) * act_scale
    out4 = (Sx @ a3.sum(0).T) * act_scale              # [B,J]
    corr34 = out3 + out4

    xtT = np.ascontiguousarray(
        Xi.reshape(B, K, N).transpose(2, 1, 0).reshape(N, K * B))
    xt8 = xtT.astype(np.int8)
    xth = np.ascontiguousarray(xtT[:, 0:HEADK * B]).astype(ml_dtypes.bfloat16)

    in_maps = []
    for cid in range(NCORES):
        jsl = slice(cid * JLOC, (cid + 1) * JLOC)
        zt = np.ascontiguousarray(
            Zs[:, jsl].transpose(4, 2, 1, 0, 3).reshape(N, K * 128)).astype(
                ml_dtypes.bfloat16)                    # [n,(k,j,p,l)]
        yt = np.ascontiguousarray(
            (c[:, jsl, :, None, None] * Ys[:, jsl]).transpose(
                1, 0, 4, 2, 3).reshape(128, K * M)).astype(np.float16)
        ut = np.zeros((UROWS, JLOC * M), np.float16)
        rt = np.zeros((UROWS, B), np.float16)
        for j in range(JLOC):
            jg = cid * JLOC + j
            ut[0:32, j * M:(j + 1) * M] = B2[jg].astype(np.float16)
            ut[32 + j, j * M:(j + 1) * M] = 1.0
            ut[36, j * M:(j + 1) * M] = bias[jg * M:(jg + 1) * M].astype(
                np.float16)
            rt[32 + j] = corr34[:, jg].astype(np.float16)
        rt[0:32] = Sx.T.astype(np.float16)
        rt[36] = 1.0
        in_maps.append({"xt8": xt8, "xth": xth, "zt": zt, "yt": yt,
                        "ut": ut, "rt": rt})
    return in_maps


def _run(inputs, trace=False):
    if "nc" not in _CACHE:
        nc = _build_bass()
        nc.finalize()          # run bacc passes (reg alloc, wait splitting)
        _CACHE["nc"] = nc
    nc = _CACHE["nc"]
    in_maps = _prepare(inputs)
    res = run_bass_kernel_spmd(nc, in_maps, list(range(NCORES)), trace=trace)
    out = np.concatenate(
        [res.results[c]["out"].astype(np.float32).T for c in range(NCORES)],
        axis=1)
    out = out.reshape(1, B, J * M)
    return out, res


def kernel(**inputs) -> np.ndarray:
    out, _ = _run(inputs, trace=False)
    return out


# revision 22
# speedup vs baseline: 1.1186x; 1.1186x over previous
"""BQQ linear inference kernel for 8 Trainium2 NeuronCores.

Factorized form: after (host-side) activation quantization the op is linear
in X_int, and each (j,k) weight block is rank-32 plus rank-2 corrections:

  out[b, (j,m)] = act_scale * sum_k [ Yc_jk^T (Z_jk X_k^T) ]{m,b}  (main)
               + corrections(Sx, out3, out4) + bias

Device kernel per core (j-sharded, 4 of 32 j-blocks):
  stage0  corrections+bias enter PSUM as one 37-row matmul per j block
          (rows: 32 Sx terms, 4 per-j scalars, 1 bias row).
  stage1  T_k[(j,p,l), b] = Zstack_k^T @ X_k^T   (one 128x512 MM per k)
  copy    T_k PSUM -> SBUF fp16, split across vector/scalar engines
  stage2  outp_j[m, b] += Yc_jk^T @ T_k[32j:32j+32]  (4 row-tiled rank-32
          MMs per k, tile_position=(32j,0), software-pipelined one k behind
          stage1 so the PE never waits on the copies)
  out     PSUM -> SBUF bf16, DMA out as [jm, b]; host transposes.

x ships int8 (2MB) and is upcast to bf16 on the otherwise idle gpsimd
engine.  Per-core HBM traffic ~5.6 MB; PE streams ~half the dense GEMM.
"""

import numpy as np
import ml_dtypes

import concourse.bass as bass
import concourse.bacc as bacc
import concourse.mybir as mybir
from concourse.tile import TileContext
from concourse.tile_rust import add_dep_helper
from concourse.bass_utils import run_bass_kernel_spmd

F32 = mybir.dt.float32
F8 = mybir.dt.float8e4
BF16 = mybir.dt.bfloat16
FP16 = mybir.dt.float16
I8 = mybir.dt.int8

P_, J, K, M, L, N = 2, 32, 32, 128, 16, 128
B = 512                  # tokens
NCORES = 8
JLOC = J // NCORES       # 4 j-blocks per core
CPJ = JLOC * M           # 512 output rows per core ((j,m) major)
QMAX = 127.0
UROWS = 37               # 32 Sx rows + 4 corr rows + 1 bias row
WARMUP = 72

_CACHE = {}


def _build_bass():
    nc = bacc.Bacc()
    xt_d = nc.declare_dram_parameter("xt8", [N, K * B], I8, isOutput=False)
    z_d = nc.declare_dram_parameter("zt", [N, K * 128], F8, isOutput=False)
    y_d = nc.declare_dram_parameter("yt", [128, K * M], F8, isOutput=False)
    c_d = nc.declare_dram_parameter("cvt", [128, K], F32, isOutput=False)
    u_d = nc.declare_dram_parameter("ut", [UROWS, JLOC * M], FP16,
                                    isOutput=False)
    r_d = nc.declare_dram_parameter("rt", [UROWS, B], FP16, isOutput=False)
    out_d = nc.declare_dram_parameter("out", [CPJ, B], BF16, isOutput=True)

    with TileContext(nc) as tc:
        with tc.tile_pool(name="big", bufs=1) as big, \
             tc.tile_pool(name="sm", bufs=1) as sm, \
             tc.tile_pool(name="ot", bufs=4) as ot, \
             tc.tile_pool(name="psum", bufs=1, space="PSUM") as pp:
            xi8 = big.tile([N, K * B], I8)        # x^T int8
            xbt = big.tile([N, K * B], BF16)      # x^T upcast to bf16
            ztt = big.tile([N, K * 128], F8)      # Z sign stacks
            ytt = big.tile([128, K * M], F8)      # Y sign stacks
            cvt = sm.tile([128, K], F32)          # per-(j,p,l) row scales
            tsb = [big.tile([128, B], FP16, name=f'tsb{i}') for i in range(3)]
            utt = sm.tile([UROWS, JLOC * M], FP16)
            rtt = sm.tile([UROWS, B], FP16)
            wz = sm.tile([128, 192], BF16)        # zeros for PE warmup
            wzms = nc.vector.memset(wz[:], 0.0)

            tp = [pp.tile([128, B], F32, name=f"tps{i}", tag=f"tps{i}")
                  for i in range(3)]
            outp = [pp.tile([128, B], F32, name=f"op{i}", tag=f"op{i}")
                    for i in range(4)]
            wps = pp.tile([128, 64], F32, name="wps", tag="wps")

            # Phase A: k-ordered streaming.  x^T int8 on the sync HWDGE
            # ring; Z/Y stacks on the scalar ring; correction matrices
            # first (they open the PSUM accumulation groups).  Slim dummy
            # matmuls paced by the first DMA warm the PE HAM clock.
            # All input DMAs issue from the (otherwise idle) sync engine in
            # one k-ordered sequence; each trigger costs ~650ns of engine
            # time, so keeping them off vector/scalar matters.
            def dx_i(ka, kb):      # int8 slices of x
                nc.sync.dma_start(out=xi8[:, ka * B:kb * B],
                                  in_=xt_d[:, ka * B:kb * B])
            def dz(ka, kb):
                nc.sync.dma_start(out=ztt[:, ka * 128:kb * 128],
                                  in_=z_d[:, ka * 128:kb * 128])
            def dy(ka, kb):
                nc.sync.dma_start(out=ytt[:, ka * M:kb * M],
                                  in_=y_d[:, ka * M:kb * M])
            nc.sync.dma_start(out=utt[:], in_=u_d[:])
            nc.sync.dma_start(out=rtt[:], in_=r_d[:])
            nc.sync.dma_start(out=cvt[:], in_=c_d[:])
            dx_i(0, 2); dz(0, 8); dy(0, 8); dx_i(2, 4)
            dx_i(4, 8); dz(8, 16); dy(8, 16); dx_i(8, 12)
            dx_i(12, 16); dz(16, 24); dy(16, 24); dx_i(16, 20)
            dx_i(20, 24); dz(24, 32); dy(24, 32); dx_i(24, 28)
            dx_i(28, 32)
            # PE warmup paced off the wz memset so it runs during the DMA
            # wait and hands the HAM clock over warm.
            for w in range(WARMUP):
                mm = nc.tensor.matmul(
                    wps[:], lhsT=wz[:, 0:128],
                    rhs=wz[:, 128:192], start=True, stop=True)
                add_dep_helper(mm.ins, wzms.ins,
                               reason="pace PE warmup after wz memset")

            # x upcast int8 -> bf16 on DVE, 2-k pieces (the scalar engine
            # faults on int8 input; gpsimd is ~10x too slow)
            def upcast(ka, kb):
                nc.vector.tensor_copy(out=xbt[:, ka * B:kb * B],
                                      in_=xi8[:, ka * B:kb * B])

            for g in range(0, 6, 2):
                upcast(g, g + 2)

            # Phase B: corrections open the output accumulation groups.
            for j in range(JLOC):
                nc.tensor.matmul(
                    outp[j][:], lhsT=utt[:, j * M:(j + 1) * M], rhs=rtt[:],
                    start=True, stop=False)

            def s1(k):
                nc.tensor.matmul(
                    tp[k % 3][:], lhsT=ztt[:, k * 128:(k + 1) * 128],
                    rhs=xbt[:, k * B:(k + 1) * B], start=True, stop=True)

            def tcopy(k):
                # PSUM -> SBUF fp16, scaled per row by c[(j,p,l), k]
                t = tsb[k % 3]
                nc.vector.tensor_scalar(out=t[:, 0:128],
                                        in0=tp[k % 3][:, 0:128],
                                        scalar1=cvt[:, k:k + 1], scalar2=None,
                                        op0=mybir.AluOpType.mult)
                nc.scalar.mul(t[:, 128:512], tp[k % 3][:, 128:512],
                              cvt[:, k:k + 1])

            def s2(k):
                for j in range(JLOC):
                    nc.tensor.matmul(
                        outp[j][:],
                        lhsT=ytt[32 * j:32 * (j + 1), k * M:(k + 1) * M],
                        rhs=tsb[k % 3][32 * j:32 * (j + 1), :],
                        start=False, stop=(k == K - 1),
                        tile_position=(32 * j, 0))

            s1(0)
            tcopy(0)
            s1(1)
            tcopy(1)
            for k in range(2, K):
                s1(k)
                s2(k - 2)
                tcopy(k)
                ku = 6 + 2 * (k - 2)
                if ku < K:
                    upcast(ku, ku + 2)
            s2(K - 2)
            s2(K - 1)

            # Phase C: PSUM -> SBUF bf16 (split engines), DMA out rows
            # (j,m); the host transposes to [b, (j,m)].
            for j in range(JLOC):
                o = ot.tile([128, B], BF16)
                nc.vector.tensor_copy(out=o[:, 0:256], in_=outp[j][:, 0:256])
                nc.scalar.copy(o[:, 256:512], outp[j][:, 256:512])
                eng = nc.sync if j % 2 == 0 else nc.scalar
                eng.dma_start(out=out_d[j * 128:(j + 1) * 128, :], in_=o[:])
    return nc


def _prepare(inputs):
    x = np.asarray(inputs["input"], dtype=np.float32)
    Ys = np.asarray(inputs["Y_sign"], np.float32)
    Zs = np.asarray(inputs["Z_sign"], np.float32)
    ysc = np.asarray(inputs["Y_scale"], np.float32)[..., 0, 0]
    zsc = np.asarray(inputs["Z_scale"], np.float32)[..., 0, 0]
    A = np.asarray(inputs["A"], np.float32)
    bias = np.asarray(inputs["bias"], np.float32)
    a0, a1, a2, a3 = A[..., 0], A[..., 1], A[..., 2], A[..., 3]

    # activation quantization on host (exact global max/min, RNE round)
    act_scale = max((float(x.max()) - float(x.min())) / (2.0 * QMAX), 1e-8)
    Xi = np.clip(np.round(x.reshape(B, K * N) / act_scale), -QMAX, QMAX)
    Xkn = Xi.reshape(B, K, N)
    Sx = Xkn.sum(-1)                                   # [B,K] (exact ints)

    c = a0 * ysc * zsc * act_scale                     # [P,J,K]
    B2 = np.einsum('pjk,pjkm->jkm', a1 * ysc, Ys.sum(-1)) * act_scale
    C3 = np.einsum('pjk,pjkn->jkn', a2 * zsc, Zs.sum(-2))
    out3 = np.einsum('bkn,jkn->bj', Xkn, C3) * act_scale
    out4 = (Sx @ a3.sum(0).T) * act_scale              # [B,J]
    corr34 = out3 + out4

    xt8 = np.ascontiguousarray(
        Xi.reshape(B, K, N).transpose(2, 1, 0).reshape(N, K * B)).astype(
            np.int8)

    in_maps = []
    for cid in range(NCORES):
        jsl = slice(cid * JLOC, (cid + 1) * JLOC)
        zt = np.ascontiguousarray(
            Zs[:, jsl].transpose(4, 2, 1, 0, 3).reshape(N, K * 128)).astype(
                ml_dtypes.float8_e4m3fn)               # [n,(k,j,p,l)] signs
        yt = np.ascontiguousarray(
            Ys[:, jsl].transpose(1, 0, 4, 2, 3).reshape(128, K * M)).astype(
                ml_dtypes.float8_e4m3fn)               # [(j,p,l),(k,m)] signs
        cvt = np.ascontiguousarray(
            np.broadcast_to(c[:, jsl, :, None],
                            (P_, JLOC, K, L)).transpose(1, 0, 3, 2).reshape(
                                128, K)).astype(np.float32)
        ut = np.zeros((UROWS, JLOC * M), np.float16)
        rt = np.zeros((UROWS, B), np.float16)
        for j in range(JLOC):
            jg = cid * JLOC + j
            ut[0:32, j * M:(j + 1) * M] = B2[jg].astype(np.float16)
            ut[32 + j, j * M:(j + 1) * M] = 1.0
            ut[36, j * M:(j + 1) * M] = bias[jg * M:(jg + 1) * M].astype(
                np.float16)
            rt[32 + j] = corr34[:, jg].astype(np.float16)
        rt[0:32] = Sx.T.astype(np.float16)
        rt[36] = 1.0
        in_maps.append({"xt8": xt8, "zt": zt, "yt": yt, "cvt": cvt,
                        "ut": ut, "rt": rt})
    return in_maps


def _run(inputs, trace=False):
    if "nc" not in _CACHE:
        nc = _build_bass()
        nc.finalize()          # run bacc passes (reg alloc, wait splitting)
        _CACHE["nc"] = nc
    nc = _CACHE["nc"]
    in_maps = _prepare(inputs)
    res = run_bass_kernel_spmd(nc, in_maps, list(range(NCORES)), trace=trace)
    out = np.concatenate(
        [res.results[c]["out"].astype(np.float32).T for c in range(NCORES)],
        axis=1)
    out = out.reshape(1, B, J * M)
    return out, res


def kernel(**inputs) -> np.ndarray:
    out, _ = _run(inputs, trace=False)
    return out


# revision 30
# speedup vs baseline: 1.1370x; 1.0164x over previous
"""BQQ linear inference kernel for 8 Trainium2 NeuronCores.

Math: after activation quantization, the whole BQQ op is linear in the
quantized input, so all four correction terms fold into one weight matrix:

    out[b, (j,m)] = X_int[b, (k,n)] @ W'[(k,n), (j,m)] + bias

where X_int = clip(round(x / act_scale), -127, 127) and W' = act_scale * W
is a pure function of the weights (Y_sign/Z_sign/scales/A) and the global
activation scale, all computed on the host (offline weight folding + act
quantization).  The device kernel per core is a pure streaming GEMM:
  1. DMA x^T (int8, upcast to bf16 on DVE) + W' shard (bf16) in, k-ordered
     with escalating chunk sizes so the GEMM starts as soon as k=0 lands.
  2. bias enters PSUM as a contraction-1 matmul (ones outer bias row) that
     opens each accumulation group.
  3. 128-contraction GEMM accumulating over k in PSUM; the last k-steps run
     bank-by-bank so each bank's epilogue overlaps the remaining matmuls.
  4. PSUM -> SBUF bf16 copies (scalar/vector split), DMA out.

Sharding: tensor-parallel over the j (output block) dim, 4 of 32 j-blocks per
core.  Per-core HBM traffic ~6.5 MB (x 2MB int8 + W 4MB bf16 + out 0.5MB).
"""

import numpy as np
import ml_dtypes

import concourse.bass as bass
import concourse.bacc as bacc
import concourse.mybir as mybir
from concourse.tile import TileContext
from concourse.tile_rust import add_dep_helper
from concourse.bass_utils import run_bass_kernel_spmd

F32 = mybir.dt.float32
BF16 = mybir.dt.bfloat16
I8 = mybir.dt.int8

P_, J, K, M, L, N = 2, 32, 32, 128, 16, 128
B = 512                  # tokens
NCORES = 8
JLOC = J // NCORES       # 4 j-blocks per core
CPJ = JLOC * M           # 512 output cols per core
QMAX = 127.0
# k-slices per DMA chunk, escalating so the GEMM k-loop starts early
CHUNKS = [1, 1, 2, 4, 4, 4, 4, 4, 4, 4]
WARMUP = 64
KSPLIT = 24              # k < KSPLIT: banks interleaved; then bank-by-bank

_CACHE = {}


def _build_bass():
    nc = bacc.Bacc()
    xt_d = nc.declare_dram_parameter("xt8", [N, K * B], I8, isOutput=False)
    w_d = nc.declare_dram_parameter("wgt", [N, K * CPJ], BF16, isOutput=False)
    b_d = nc.declare_dram_parameter("bias", [1, CPJ], BF16, isOutput=False)
    out_d = nc.declare_dram_parameter("out", [B, CPJ], BF16, isOutput=True)

    with TileContext(nc) as tc:
        with tc.tile_pool(name="big", bufs=1) as big, \
             tc.tile_pool(name="sm", bufs=1) as sm, \
             tc.tile_pool(name="ot", bufs=4) as ot, \
             tc.tile_pool(name="psum", bufs=1, space="PSUM") as pp:
            xi8 = big.tile([N, K * B], I8)        # x^T int8
            xbt = big.tile([N, K * B], BF16)      # x^T upcast to bf16
            wt = big.tile([N, K * CPJ], BF16)     # folded weights
            wz = sm.tile([128, 192], BF16)        # zeros for PE warmup
            ones_r = sm.tile([1, 128], BF16)
            bias_t = sm.tile([1, CPJ], BF16)
            nc.vector.memset(wz[:], 0.0)
            nc.vector.memset(ones_r[:], 1.0)

            psums = [pp.tile([128, CPJ], F32, name=f"psum{i}", tag=f"psum{i}")
                     for i in range(4)]
            wps = pp.tile([128, 64], F32, name="wps", tag="wps")

            # Phase A: stream x^T (sync HWDGE ring) and weights (scalar HWDGE
            # ring) in parallel, k-ordered; upcast each x chunk on DVE as it
            # lands.  A long run of slim dummy matmuls paced by the first DMA
            # trigger keeps the PE busy through the HAM window so the GEMM
            # starts at full clock.
            bdma = nc.gpsimd.dma_start(out=bias_t[:], in_=b_d[:])
            k0 = 0
            for ci, nk in enumerate(CHUNKS):
                xsl = slice(k0 * B, (k0 + nk) * B)
                wsl = slice(k0 * CPJ, (k0 + nk) * CPJ)
                dma = nc.sync.dma_start(out=xi8[:, xsl], in_=xt_d[:, xsl])
                nc.scalar.dma_start(out=wt[:, wsl], in_=w_d[:, wsl])
                for kk in range(k0, k0 + nk):
                    nc.vector.tensor_copy(out=xbt[:, kk * B:(kk + 1) * B],
                                          in_=xi8[:, kk * B:(kk + 1) * B])
                if ci == 0:
                    for w in range(WARMUP):
                        mm = nc.tensor.matmul(
                            wps[:], lhsT=wz[:, 0:128],
                            rhs=wz[:, 128:192], start=True, stop=True)
                        add_dep_helper(mm.ins, bdma.ins,
                                       reason="pace PE warmup with bias DMA")
                k0 += nk

            # Phase B: bias opens each accumulation group (contraction-1
            # outer product ones x bias_row), then the GEMM k-loop.  The
            # last K - KSPLIT steps run bank-by-bank so bank bb's epilogue
            # can start while bank bb+1 is still accumulating.
            def mm_step(k, bb, stop):
                nc.tensor.matmul(
                    psums[bb][:],
                    lhsT=xbt[:, k * B + bb * 128:k * B + (bb + 1) * 128],
                    rhs=wt[:, k * CPJ:(k + 1) * CPJ],
                    start=False, stop=stop)

            def epilogue(bb):
                o = ot.tile([128, CPJ], BF16)
                if bb % 2 == 0:
                    nc.scalar.copy(o[:], psums[bb][:])
                else:
                    nc.vector.tensor_copy(out=o[:], in_=psums[bb][:])
                eng = nc.sync if bb % 2 == 0 else nc.scalar
                eng.dma_start(out=out_d[bb * 128:(bb + 1) * 128, :], in_=o[:])

            for bb in range(4):
                nc.tensor.matmul(
                    psums[bb][:], lhsT=ones_r[:], rhs=bias_t[:],
                    start=True, stop=False)
            for k in range(KSPLIT):
                for bb in range(4):
                    mm_step(k, bb, stop=False)
            for bb in range(4):
                for k in range(KSPLIT, K):
                    mm_step(k, bb, stop=(k == K - 1))
                epilogue(bb)
    return nc


def _fold_weights(Y_sign, Z_sign, Y_scale, Z_scale, A):
    """W[j,k,n,m]: everything linear in X folded into one matrix (fp32)."""
    ysc = Y_scale[..., 0, 0].astype(np.float32)      # (p,j,k)
    zsc = Z_scale[..., 0, 0].astype(np.float32)
    a0, a1, a2, a3 = (A[..., i].astype(np.float32) for i in range(4))
    Zs = Z_sign.astype(np.float32)
    Ys = Y_sign.astype(np.float32)
    # out1: sum_{p,l} a0*ysc*zsc * Z[l,n] * Y[m,l]  -> (j,k,n,m)
    t1 = np.einsum('pjkln,pjkml->pjknm', Zs, Ys, optimize=True)
    W = np.einsum('pjk,pjknm->jknm', a0 * ysc * zsc, t1, optimize=True)
    # out2: B_coef[j,k,m] broadcast over n
    Ysum = Ys.sum(-1) * ysc[..., None]               # (p,j,k,m)
    W += np.einsum('pjk,pjkm->jkm', a1, Ysum)[:, :, None, :]
    # out3: sum_p a2*zsc*Zsum[n] broadcast over m
    Zsum = Zs.sum(-2) * zsc[..., None]               # (p,j,k,n)
    W += np.einsum('pjk,pjkn->jkn', a2, Zsum)[:, :, :, None]
    # out4: D_coef[j,k] broadcast over n,m
    W += a3.sum(0)[:, :, None, None]
    return W


def _prepare(inputs):
    x = np.asarray(inputs["input"], dtype=np.float32)
    W = _fold_weights(np.asarray(inputs["Y_sign"], np.float32),
                      np.asarray(inputs["Z_sign"], np.float32),
                      np.asarray(inputs["Y_scale"], np.float32),
                      np.asarray(inputs["Z_scale"], np.float32),
                      np.asarray(inputs["A"], np.float32))
    bias = np.asarray(inputs["bias"], np.float32)

    # activation quantization on host (exact global max/min, RNE round)
    act_scale = max((float(x.max()) - float(x.min())) / (2.0 * QMAX), 1e-8)
    xq = np.clip(np.round(x / act_scale), -QMAX, QMAX)
    W = W * act_scale    # fold act_scale into the weights

    # x^T layout [n, (k, b)], int8
    xt8 = np.ascontiguousarray(
        xq.reshape(B, K, N).transpose(2, 1, 0).reshape(N, K * B)).astype(
            np.int8)

    in_maps = []
    for cid in range(NCORES):
        Wc = W[cid * JLOC:(cid + 1) * JLOC]          # [jl,k,n,m]
        wgt = np.ascontiguousarray(
            Wc.transpose(2, 1, 0, 3).reshape(N, K * CPJ)).astype(
                ml_dtypes.bfloat16)                  # [n, (k, jl, m)]
        bc = np.ascontiguousarray(
            bias[cid * CPJ:(cid + 1) * CPJ].reshape(1, CPJ)).astype(
                ml_dtypes.bfloat16)
        in_maps.append({"xt8": xt8, "wgt": wgt, "bias": bc})
    return in_maps


def _run(inputs, trace=False):
    if "nc" not in _CACHE:
        nc = _build_bass()
        nc.finalize()          # run bacc passes (reg alloc, wait splitting)
        _CACHE["nc"] = nc
    nc = _CACHE["nc"]
    in_maps = _prepare(inputs)
    res = run_bass_kernel_spmd(nc, in_maps, list(range(NCORES)), trace=trace)
    out = np.concatenate([res.results[c]["out"].astype(np.float32)
                          for c in range(NCORES)], axis=1)
    out = out.reshape(1, B, J * M)
    return out, res


def kernel(**inputs) -> np.ndarray:
    out, _ = _run(inputs, trace=False)
    return out


# revision 31
# speedup vs baseline: 1.1866x; 1.0437x over previous
"""BQQ linear inference kernel for 8 Trainium2 NeuronCores.

Math: after activation quantization, the whole BQQ op is linear in the
quantized input, so all four correction terms fold into one weight matrix:

    out[b, (j,m)] = X_int[b, (k,n)] @ W'[(k,n), (j,m)] + bias

where X_int = clip(round(x / act_scale), -127, 127) and W' = act_scale * W
is a pure function of the weights (Y_sign/Z_sign/scales/A) and the global
activation scale, all computed on the host (offline weight folding + act
quantization).  The device kernel per core is a pure streaming GEMM:
  1. DMA x^T (int8, upcast to bf16 on DVE) + W' shard (bf16) in, k-ordered
     with escalating chunk sizes so the GEMM starts as soon as k=0 lands.
  2. bias enters PSUM as a contraction-1 matmul (ones outer bias row) that
     opens each accumulation group.
  3. 128-contraction GEMM accumulating over k in PSUM; the last k-steps run
     bank-by-bank so each bank's epilogue overlaps the remaining matmuls.
  4. PSUM -> SBUF bf16 copies (scalar/vector split), DMA out.

Sharding: tensor-parallel over the j (output block) dim, 4 of 32 j-blocks per
core.  Per-core HBM traffic ~6.5 MB (x 2MB int8 + W 4MB bf16 + out 0.5MB).
"""

import numpy as np
import ml_dtypes

import concourse.bass as bass
import concourse.bacc as bacc
import concourse.mybir as mybir
from concourse.tile import TileContext
from concourse.tile_rust import add_dep_helper
from concourse.bass_utils import run_bass_kernel_spmd

F32 = mybir.dt.float32
BF16 = mybir.dt.bfloat16
I8 = mybir.dt.int8

P_, J, K, M, L, N = 2, 32, 32, 128, 16, 128
B = 512                  # tokens
NCORES = 8
JLOC = J // NCORES       # 4 j-blocks per core
CPJ = JLOC * M           # 512 output cols per core
QMAX = 127.0
# k-slices per DMA chunk, escalating so the GEMM k-loop starts early
CHUNKS = [1, 1, 2, 4, 4, 4, 4, 4, 4, 4]
WARMUP = 56
KSPLIT = 24              # k < KSPLIT: banks interleaved; then bank-by-bank

_CACHE = {}


def _build_bass():
    nc = bacc.Bacc()
    xt_d = nc.declare_dram_parameter("xt8", [N, K * B], I8, isOutput=False)
    w_d = nc.declare_dram_parameter("wgt", [N, K * CPJ], BF16, isOutput=False)
    b_d = nc.declare_dram_parameter("bias", [1, CPJ], BF16, isOutput=False)
    out_d = nc.declare_dram_parameter("out", [B, CPJ], BF16, isOutput=True)

    with TileContext(nc) as tc:
        with tc.tile_pool(name="big", bufs=1) as big, \
             tc.tile_pool(name="sm", bufs=1) as sm, \
             tc.tile_pool(name="ot", bufs=4) as ot, \
             tc.tile_pool(name="psum", bufs=1, space="PSUM") as pp:
            xi8 = big.tile([N, K * B], I8)        # x^T int8
            xbt = big.tile([N, K * B], BF16)      # x^T upcast to bf16
            wt = big.tile([N, K * CPJ], BF16)     # folded weights
            wz = sm.tile([128, 192], BF16)        # zeros for PE warmup
            ones_r = sm.tile([1, 128], BF16)
            bias_t = sm.tile([1, CPJ], BF16)
            wzms = nc.vector.memset(wz[:], 0.0)
            nc.vector.memset(ones_r[:], 1.0)

            psums = [pp.tile([128, CPJ], F32, name=f"psum{i}", tag=f"psum{i}")
                     for i in range(4)]
            wps = pp.tile([128, 64], F32, name="wps", tag="wps")

            # Phase A: stream x^T (sync HWDGE ring) and weights (scalar HWDGE
            # ring) in parallel, k-ordered; upcast each x chunk on DVE as it
            # lands.  A long run of slim dummy matmuls paced by the first DMA
            # trigger keeps the PE busy through the HAM window so the GEMM
            # starts at full clock.
            bdma = nc.gpsimd.dma_start(out=bias_t[:], in_=b_d[:])
            k0 = 0
            for ci, nk in enumerate(CHUNKS):
                xsl = slice(k0 * B, (k0 + nk) * B)
                wsl = slice(k0 * CPJ, (k0 + nk) * CPJ)
                dma = nc.sync.dma_start(out=xi8[:, xsl], in_=xt_d[:, xsl])
                nc.scalar.dma_start(out=wt[:, wsl], in_=w_d[:, wsl])
                for kk in range(k0, k0 + nk):
                    nc.vector.tensor_copy(out=xbt[:, kk * B:(kk + 1) * B],
                                          in_=xi8[:, kk * B:(kk + 1) * B])
                if ci == 0:
                    for w in range(WARMUP):
                        mm = nc.tensor.matmul(
                            wps[:], lhsT=wz[:, 0:128],
                            rhs=wz[:, 128:192], start=True, stop=True)
                        add_dep_helper(mm.ins, wzms.ins,
                                       reason="pace PE warmup after memset")
                k0 += nk

            # Phase B: bias opens each accumulation group (contraction-1
            # outer product ones x bias_row), then the GEMM k-loop.  The
            # last K - KSPLIT steps run bank-by-bank so bank bb's epilogue
            # can start while bank bb+1 is still accumulating.
            def mm_step(k, bb, stop):
                nc.tensor.matmul(
                    psums[bb][:],
                    lhsT=xbt[:, k * B + bb * 128:k * B + (bb + 1) * 128],
                    rhs=wt[:, k * CPJ:(k + 1) * CPJ],
                    start=False, stop=stop)

            def epilogue(bb):
                o = ot.tile([128, CPJ], BF16)
                if bb % 2 == 0:
                    nc.scalar.copy(o[:], psums[bb][:])
                else:
                    nc.vector.tensor_copy(out=o[:], in_=psums[bb][:])
                eng = nc.sync if bb % 2 == 0 else nc.scalar
                eng.dma_start(out=out_d[bb * 128:(bb + 1) * 128, :], in_=o[:])

            for bb in range(4):
                nc.tensor.matmul(
                    psums[bb][:], lhsT=ones_r[:], rhs=bias_t[:],
                    start=True, stop=False)
            for k in range(KSPLIT):
                for bb in range(4):
                    mm_step(k, bb, stop=False)
            for bb in range(4):
                for k in range(KSPLIT, K):
                    mm_step(k, bb, stop=(k == K - 1))
                epilogue(bb)
    return nc


def _fold_weights(Y_sign, Z_sign, Y_scale, Z_scale, A):
    """W[j,k,n,m]: everything linear in X folded into one matrix (fp32)."""
    ysc = Y_scale[..., 0, 0].astype(np.float32)      # (p,j,k)
    zsc = Z_scale[..., 0, 0].astype(np.float32)
    a0, a1, a2, a3 = (A[..., i].astype(np.float32) for i in range(4))
    Zs = Z_sign.astype(np.float32)
    Ys = Y_sign.astype(np.float32)
    # out1: sum_{p,l} a0*ysc*zsc * Z[l,n] * Y[m,l]  -> (j,k,n,m)
    t1 = np.einsum('pjkln,pjkml->pjknm', Zs, Ys, optimize=True)
    W = np.einsum('pjk,pjknm->jknm', a0 * ysc * zsc, t1, optimize=True)
    # out2: B_coef[j,k,m] broadcast over n
    Ysum = Ys.sum(-1) * ysc[..., None]               # (p,j,k,m)
    W += np.einsum('pjk,pjkm->jkm', a1, Ysum)[:, :, None, :]
    # out3: sum_p a2*zsc*Zsum[n] broadcast over m
    Zsum = Zs.sum(-2) * zsc[..., None]               # (p,j,k,n)
    W += np.einsum('pjk,pjkn->jkn', a2, Zsum)[:, :, :, None]
    # out4: D_coef[j,k] broadcast over n,m
    W += a3.sum(0)[:, :, None, None]
    return W


def _prepare(inputs):
    x = np.asarray(inputs["input"], dtype=np.float32)
    W = _fold_weights(np.asarray(inputs["Y_sign"], np.float32),
                      np.asarray(inputs["Z_sign"], np.float32),
                      np.asarray(inputs["Y_scale"], np.float32),
                      np.asarray(inputs["Z_scale"], np.float32),
                      np.asarray(inputs["A"], np.float32))
    bias = np.asarray(inputs["bias"], np.float32)

    # activation quantization on host (exact global max/min, RNE round)
    act_scale = max((float(x.max()) - float(x.min())) / (2.0 * QMAX), 1e-8)
    xq = np.clip(np.round(x / act_scale), -QMAX, QMAX)
    W = W * act_scale    # fold act_scale into the weights

    # x^T layout [n, (k, b)], int8
    xt8 = np.ascontiguousarray(
        xq.reshape(B, K, N).transpose(2, 1, 0).reshape(N, K * B)).astype(
            np.int8)

    in_maps = []
    for cid in range(NCORES):
        Wc = W[cid * JLOC:(cid + 1) * JLOC]          # [jl,k,n,m]
        wgt = np.ascontiguousarray(
            Wc.transpose(2, 1, 0, 3).reshape(N, K * CPJ)).astype(
                ml_dtypes.bfloat16)                  # [n, (k, jl, m)]
        bc = np.ascontiguousarray(
            bias[cid * CPJ:(cid + 1) * CPJ].reshape(1, CPJ)).astype(
                ml_dtypes.bfloat16)
        in_maps.append({"xt8": xt8, "wgt": wgt, "bias": bc})
    return in_maps


def _run(inputs, trace=False):
    if "nc" not in _CACHE:
        nc = _build_bass()
        nc.finalize()          # run bacc passes (reg alloc, wait splitting)
        _CACHE["nc"] = nc
    nc = _CACHE["nc"]
    in_maps = _prepare(inputs)
    res = run_bass_kernel_spmd(nc, in_maps, list(range(NCORES)), trace=trace)
    out = np.concatenate([res.results[c]["out"].astype(np.float32)
                          for c in range(NCORES)], axis=1)
    out = out.reshape(1, B, J * M)
    return out, res


def kernel(**inputs) -> np.ndarray:
    out, _ = _run(inputs, trace=False)
    return out


# revision 46
# speedup vs baseline: 1.2386x; 1.0438x over previous
"""BQQ linear inference kernel for 8 Trainium2 NeuronCores.

Math: after activation quantization, the whole BQQ op is linear in the
quantized input, so all four correction terms fold into one weight matrix:

    out[b, (j,m)] = X_int[b, (k,n)] @ W'[(k,n), (j,m)] + bias

where X_int = clip(round(x / act_scale), -127, 127) and W' = act_scale * W
is a pure function of the weights (Y_sign/Z_sign/scales/A) and the global
activation scale, all computed on the host (offline weight folding + act
quantization).  The device kernel per core is a pure streaming GEMM:
  1. DMA k0 of x^T as ready bf16, the rest as int8 (upcast to bf16 on
     DVE) + W' shard (bf16), k-ordered with escalating chunk sizes so the
     GEMM starts as soon as k=0 lands; warmup matmuls paced off an early
     memset hold the HAM clock warm through the DMA wait.
  2. 128-contraction GEMM accumulating over k in PSUM; the last k-steps run
     bank-by-bank so each bank's epilogue overlaps the remaining matmuls.
  3. Epilogue per bank: DVE adds the broadcast bias while casting
     PSUM -> SBUF bf16 (bias DMA rides the weight ring, needed only at the
     tail), then DMA out in column-halves on both by-then-idle input
     rings so the final transfers and completion receipts run in parallel.

Sharding: tensor-parallel over the j (output block) dim, 4 of 32 j-blocks per
core.  Per-core HBM traffic ~6.5 MB (x 2MB int8 + W 4MB bf16 + out 0.5MB).
"""

import numpy as np
import ml_dtypes

import concourse.bass as bass
import concourse.bacc as bacc
import concourse.mybir as mybir
from concourse.tile import TileContext
from concourse.tile_rust import add_dep_helper
from concourse.bass_utils import run_bass_kernel_spmd

F32 = mybir.dt.float32
BF16 = mybir.dt.bfloat16
I8 = mybir.dt.int8

P_, J, K, M, L, N = 2, 32, 32, 128, 16, 128
B = 512                  # tokens
NCORES = 8
JLOC = J // NCORES       # 4 j-blocks per core
CPJ = JLOC * M           # 512 output cols per core
QMAX = 127.0
# k-slices per DMA chunk, escalating so the GEMM k-loop starts early
CHUNKS = [1, 1, 1, 2, 2, 4, 4, 4, 4, 4, 4]   # int8 x chunks for k >= 1
WARMUP = 64
KSPLIT = 24              # k < KSPLIT: banks interleaved; then bank-by-bank

_CACHE = {}


def _build_bass():
    nc = bacc.Bacc()
    xt_d = nc.declare_dram_parameter("xt8", [N, K * B], I8, isOutput=False)
    xh_d = nc.declare_dram_parameter("xth", [N, B], BF16, isOutput=False)
    w_d = nc.declare_dram_parameter("wgt", [N, K * CPJ], BF16, isOutput=False)
    b_d = nc.declare_dram_parameter("bias", [128, CPJ], BF16, isOutput=False)
    out_d = nc.declare_dram_parameter("out", [B, CPJ], BF16, isOutput=True)

    with TileContext(nc) as tc:
        with tc.tile_pool(name="big", bufs=1) as big, \
             tc.tile_pool(name="sm", bufs=1) as sm, \
             tc.tile_pool(name="ot", bufs=4) as ot, \
             tc.tile_pool(name="psum", bufs=1, space="PSUM") as pp:
            xi8 = big.tile([N, K * B], I8)        # x^T int8
            xbt = big.tile([N, K * B], BF16)      # x^T upcast to bf16
            wt = big.tile([N, K * CPJ], BF16)     # folded weights
            wz = sm.tile([128, 192], BF16)        # zeros for PE warmup
            bias_bc = sm.tile([128, CPJ], BF16)   # broadcast bias rows
            wzms = nc.gpsimd.memset(wz[:], 0.0)

            psums = [pp.tile([128, CPJ], F32, name=f"psum{i}", tag=f"psum{i}")
                     for i in range(4)]
            wps = pp.tile([128, 64], F32, name="wps", tag="wps")

            # Phase A: stream x^T (sync HWDGE ring) and weights (scalar HWDGE
            # ring) in parallel, k-ordered; upcast each x chunk on DVE as it
            # lands.  A long run of slim dummy matmuls paced by the first DMA
            # trigger keeps the PE busy through the HAM window so the GEMM
            # starts at full clock.
            # k0 ships as ready bf16 (no upcast on the critical first MMs)
            nc.sync.dma_start(out=xbt[:, 0:B], in_=xh_d[:])
            nc.scalar.dma_start(out=wt[:, 0:CPJ], in_=w_d[:, 0:CPJ])
            for w in range(WARMUP):
                mm = nc.tensor.matmul(
                    wps[:], lhsT=wz[:, 0:128],
                    rhs=wz[:, 128:192], start=True, stop=True)
                add_dep_helper(mm.ins, wzms.ins,
                               reason="pace PE warmup after memset")
            k0 = 1
            for nk in CHUNKS:
                xsl = slice(k0 * B, (k0 + nk) * B)
                wsl = slice(k0 * CPJ, (k0 + nk) * CPJ)
                nc.sync.dma_start(out=xi8[:, xsl], in_=xt_d[:, xsl])
                nc.scalar.dma_start(out=wt[:, wsl], in_=w_d[:, wsl])
                for kk in range(k0, k0 + nk):
                    nc.vector.tensor_copy(out=xbt[:, kk * B:(kk + 1) * B],
                                          in_=xi8[:, kk * B:(kk + 1) * B])
                k0 += nk
            # bias (needed only at the tail) rides the weight ring last
            nc.scalar.dma_start(out=bias_bc[:], in_=b_d[:])

            # Phase B: the GEMM k-loop.  The last K - KSPLIT steps run
            # bank-by-bank so bank bb's epilogue (bias add + bf16 cast)
            # can start while bank bb+1 is still accumulating.
            def mm_step(k, bb, stop):
                nc.tensor.matmul(
                    psums[bb][:],
                    lhsT=xbt[:, k * B + bb * 128:k * B + (bb + 1) * 128],
                    rhs=wt[:, k * CPJ:(k + 1) * CPJ],
                    start=(k == 0), stop=stop)

            def epilogue(bb):
                o = ot.tile([128, CPJ], BF16)
                nc.vector.tensor_add(o[:], psums[bb][:], bias_bc[:])
                rows = slice(bb * 128, (bb + 1) * 128)
                nc.sync.dma_start(out=out_d[rows, 0:256], in_=o[:, 0:256])
                nc.scalar.dma_start(out=out_d[rows, 256:512],
                                    in_=o[:, 256:512])

            for k in range(KSPLIT):
                for bb in range(4):
                    mm_step(k, bb, stop=False)
            for bb in range(4):
                for k in range(KSPLIT, K):
                    mm_step(k, bb, stop=(k == K - 1))
                epilogue(bb)
    return nc


def _fold_weights(Y_sign, Z_sign, Y_scale, Z_scale, A):
    """W[j,k,n,m]: everything linear in X folded into one matrix (fp32)."""
    ysc = Y_scale[..., 0, 0].astype(np.float32)      # (p,j,k)
    zsc = Z_scale[..., 0, 0].astype(np.float32)
    a0, a1, a2, a3 = (A[..., i].astype(np.float32) for i in range(4))
    Zs = Z_sign.astype(np.float32)
    Ys = Y_sign.astype(np.float32)
    # out1: sum_{p,l} a0*ysc*zsc * Z[l,n] * Y[m,l]  -> (j,k,n,m)
    t1 = np.einsum('pjkln,pjkml->pjknm', Zs, Ys, optimize=True)
    W = np.einsum('pjk,pjknm->jknm', a0 * ysc * zsc, t1, optimize=True)
    # out2: B_coef[j,k,m] broadcast over n
    Ysum = Ys.sum(-1) * ysc[..., None]               # (p,j,k,m)
    W += np.einsum('pjk,pjkm->jkm', a1, Ysum)[:, :, None, :]
    # out3: sum_p a2*zsc*Zsum[n] broadcast over m
    Zsum = Zs.sum(-2) * zsc[..., None]               # (p,j,k,n)
    W += np.einsum('pjk,pjkn->jkn', a2, Zsum)[:, :, :, None]
    # out4: D_coef[j,k] broadcast over n,m
    W += a3.sum(0)[:, :, None, None]
    return W


def _prepare(inputs):
    x = np.asarray(inputs["input"], dtype=np.float32)
    W = _fold_weights(np.asarray(inputs["Y_sign"], np.float32),
                      np.asarray(inputs["Z_sign"], np.float32),
                      np.asarray(inputs["Y_scale"], np.float32),
                      np.asarray(inputs["Z_scale"], np.float32),
                      np.asarray(inputs["A"], np.float32))
    bias = np.asarray(inputs["bias"], np.float32)

    # activation quantization on host (exact global max/min, RNE round)
    act_scale = max((float(x.max()) - float(x.min())) / (2.0 * QMAX), 1e-8)
    xq = np.clip(np.round(x / act_scale), -QMAX, QMAX)
    W = W * act_scale    # fold act_scale into the weights

    # x^T layout [n, (k, b)], int8
    xtT = np.ascontiguousarray(
        xq.reshape(B, K, N).transpose(2, 1, 0).reshape(N, K * B))
    xt8 = xtT.astype(np.int8)
    xth = np.ascontiguousarray(xtT[:, 0:B]).astype(ml_dtypes.bfloat16)

    in_maps = []
    for cid in range(NCORES):
        Wc = W[cid * JLOC:(cid + 1) * JLOC]          # [jl,k,n,m]
        wgt = np.ascontiguousarray(
            Wc.transpose(2, 1, 0, 3).reshape(N, K * CPJ)).astype(
                ml_dtypes.bfloat16)                  # [n, (k, jl, m)]
        bc = np.ascontiguousarray(np.broadcast_to(
            bias[cid * CPJ:(cid + 1) * CPJ].reshape(1, CPJ),
            (128, CPJ))).astype(ml_dtypes.bfloat16)
        in_maps.append({"xt8": xt8, "xth": xth, "wgt": wgt, "bias": bc})
    return in_maps


def _run(inputs, trace=False):
    if "nc" not in _CACHE:
        nc = _build_bass()
        nc.finalize()          # run bacc passes (reg alloc, wait splitting)
        _CACHE["nc"] = nc
    nc = _CACHE["nc"]
    in_maps = _prepare(inputs)
    res = run_bass_kernel_spmd(nc, in_maps, list(range(NCORES)), trace=trace)
    out = np.concatenate([res.results[c]["out"].astype(np.float32)
                          for c in range(NCORES)], axis=1)
    out = out.reshape(1, B, J * M)
    return out, res


def kernel(**inputs) -> np.ndarray:
    out, _ = _run(inputs, trace=False)
    return out
